# revision 1
# baseline (speedup 1.0000x reference)
"""Trainium2 Bass kernel v2.2: dual-softmax ("contrast") multi-head self-attention.

Problem (per full input):
  x, y: (4, 1024, 1024) f32; Wq/Wk/Wv: (1024, 1024) f32, nh=16 heads, dk=dv=64.
  dist   = softmax(q k^T / 8)
  c_att  = softmax(1 - dist) @ v      (== softmax(-dist) @ v)
  att    = softmax(dist) @ v
Sharding: 8 cores = 4 batches x 2 head-groups (8 heads each).

Design (v2.2):
  * Everything bf16 on the PE except the S^T matmuls (f32r from f32 Q^T/K^T).
  * S is computed PRE-TRANSPOSED: ST[k,q] via lhsT=KT-block, rhs=QT, so the
    softmax weights never need a PE transpose.
  * Z1[q] = sum_k exp(ST/8) via ones-vector matmuls accumulated in PSUM;
    r1 = approx-recip (1-instr DVE custom, ~51 ULP); broadcast across
    partitions with the (nearly free) Pool partition_broadcast.
  * dist^T = e1t * r1 in bf16, split DVE/Pool. e3t = exp(dist^T) [ACT],
    e2t = approx-recip(e3t) [DVE], both bf16.
  * O matmuls in DIRECT output form: [q, dv+1] accumulating over k with the
    softmax-weight block as bf16 stationary; V_aug's ones column gives the
    branch denominators. Both branches share one PSUM bank: the first matmul's
    start=True zeroes the whole 2KB zero-region, every later matmul
    accumulates (start=False) -- valid on HW where PSUM start zeroes the full
    region.
  * 3-deep head pipeline with the cross-iteration critical chain
    (Z1 -> r1 -> broadcast -> dist -> e3t) hoisted to the FRONT of each
    engine queue, and slack work (e2t, output norms) pushed to the back.
"""

import sys

if "/opt/trn_rl_repo" not in sys.path:
    sys.path.insert(0, "/opt/trn_rl_repo")

from contextlib import ExitStack

import numpy as np

import concourse.bass as bass
from concourse import bacc, masks, mybir
from concourse.bass_utils import run_bass_kernel_spmd
from concourse.dve_ops import RECIP_APPROX_FAST_CONSTS, RECIPROCAL_APPROX_FAST
from concourse.tile import TileContext

F32 = mybir.dt.float32
F32R = mybir.dt.float32r
BF16 = mybir.dt.bfloat16
EXP = mybir.ActivationFunctionType.Exp

P = 128          # partitions
N = 1024         # tokens
D = 1024         # model dim
NF = 512         # features per core (8 heads x 64)
FH = 8           # heads per core
DK = 64          # head dim
NPT = N // P     # 8 token ptiles
KBN = D // P     # 8 contraction blocks
MB = NF // P     # 4 feature ptiles

DIST_POOL_KBS = (5, 7)   # dist multiplies routed to Pool vs DVE (late beats
                         # only: Pool's tensor_mul is ~3x slower than DVE's
                         # and must not gate the ACT e3t chain at iter start)


def _r(ap):
    return ap.bitcast(F32R)


def _recip_fast(nc, out, in0):
    """~51-ULP approximate reciprocal, 1 DVE instruction (the bit-exact
    nc.vector.reciprocal runs ~6 cycles/element -- far too slow here)."""
    cc = RECIP_APPROX_FAST_CONSTS
    nc.vector._custom_dve(
        RECIPROCAL_APPROX_FAST, out=out, in0=in0,
        s0=cc["s0"], s1=cc["s1"], imm2=cc["imm2"],
    )


def build_nc():
    nc = bacc.Bacc("TRN2")
    x_d = nc.dram_tensor("x", [N, D], F32, kind="ExternalInput")
    y_d = nc.dram_tensor("y", [N, D], F32, kind="ExternalInput")
    wq_d = nc.dram_tensor("wq", [NF, D], F32, kind="ExternalInput")
    wk_d = nc.dram_tensor("wk", [NF, D], F32, kind="ExternalInput")
    wv_d = nc.dram_tensor("wv", [NF, D], F32, kind="ExternalInput")
    catt_d = nc.dram_tensor("catt", [N, NF], F32, kind="ExternalOutput")
    att_d = nc.dram_tensor("att", [N, NF], F32, kind="ExternalOutput")

    with TileContext(nc) as tc, ExitStack() as ctx:
        persist = ctx.enter_context(tc.tile_pool(name="persist", bufs=1))
        ident = persist.tile([P, P], F32)
        masks.make_identity(nc, ident[:])
        ident_b = persist.tile([P, P], BF16)
        nc.vector.tensor_copy(ident_b[:], ident[:])

        qt = persist.tile([P, MB, N], BF16)       # Q^T: [feat%128, featblk, tok]
        kt = persist.tile([P, MB, N], BF16)
        vv = persist.tile([P, NPT, FH, DK + 1], BF16)  # V_aug per head
        att_sb = persist.tile([P, NPT, NF], F32)
        catt_sb = persist.tile([P, NPT, NF], F32)
        ones_col = persist.tile([P, 1], BF16)
        nc.vector.memset(ones_col[:], 1.0)
        nc.vector.memset(vv[:, :, :, DK:DK + 1], 1.0)

        # ---------------- setup: casts + transposes + projections ---------
        with ExitStack() as sctx:
            sbp = sctx.enter_context(tc.tile_pool(name="setup", bufs=1))
            pst = sctx.enter_context(tc.tile_pool(name="pst", bufs=4, space="PSUM"))

            xt = sbp.tile([P, KBN, N], BF16, tag="xt")
            yt = sbp.tile([P, KBN, N], BF16, tag="yt")
            xb = sbp.tile([P, NPT, D], BF16, tag="xb")
            yb = sbp.tile([P, NPT, D], BF16, tag="yb")

            def copy_ps(idx, dst, src):
                if idx % 2 == 0:
                    nc.vector.tensor_copy(dst, src)
                else:
                    nc.scalar.copy(dst, src)

            # DMA f32 ptiles through small rings; down-cast to bf16 on
            # ACT (x) / DVE (y) so the PE transposes run at 1 cyc/row.
            for src_d, dst_b, eng, tag in (
                (x_d, xb, nc.scalar, "rx"), (y_d, yb, nc.vector, "ry"),
            ):
                for i in range(NPT):
                    raw = sbp.tile([P, D], F32, tag=tag, bufs=2, name="raw")
                    nc.sync.dma_start(out=raw[:], in_=src_d[i * P:(i + 1) * P, :])
                    if eng is nc.scalar:
                        eng.copy(dst_b[:, i, :], raw[:])
                    else:
                        eng.tensor_copy(dst_b[:, i, :], raw[:])

            cidx = 0
            for src_b, dst in ((xb, xt), (yb, yt)):
                for kb in range(KBN):
                    for half in range(2):
                        tp = pst.tile([P, 512], BF16, tag="tp")
                        for j in range(4):
                            i = half * 4 + j
                            nc.tensor.transpose(
                                tp[:, j * P:(j + 1) * P],
                                src_b[:, i, kb * P:(kb + 1) * P],
                                ident_b[:],
                            )
                        copy_ps(cidx, dst[:, kb, half * 512:(half + 1) * 512],
                                tp[:])
                        cidx += 1

            def load_wt(w_d, eng):
                wb = sbp.tile([P, MB, D], BF16, tag="wb", bufs=2, name="wb")
                for m in range(MB):
                    wraw = sbp.tile([P, D], F32, tag="wr", bufs=2, name="wraw")
                    nc.sync.dma_start(out=wraw[:], in_=w_d[m * P:(m + 1) * P, :])
                    if eng is nc.scalar:
                        eng.copy(wb[:, m, :], wraw[:])
                    else:
                        eng.tensor_copy(wb[:, m, :], wraw[:])
                wt = sbp.tile([P, KBN, 512], BF16, tag="wt", bufs=2)
                for kb in range(KBN):
                    tp = pst.tile([P, 512], BF16, tag="tp")
                    for m in range(MB):
                        nc.tensor.transpose(
                            tp[:, m * P:(m + 1) * P],
                            wb[:, m, kb * P:(kb + 1) * P],
                            ident_b[:],
                        )
                    copy_ps(kb, wt[:, kb, :], tp[:])
                return wt

            for w_d, out_sb, ceng in ((wq_d, qt, nc.vector), (wk_d, kt, nc.scalar)):
                wt = load_wt(w_d, ceng)
                for m in range(MB):
                    q_ps = pst.tile([P, N], F32, tag="proj", bufs=2)
                    for ch in range(2):
                        for kb in range(KBN):
                            nc.tensor.matmul(
                                q_ps[:, ch * 512:(ch + 1) * 512],
                                lhsT=wt[:, kb, m * P:(m + 1) * P],
                                rhs=xt[:, kb, ch * 512:(ch + 1) * 512],
                                start=(kb == 0),
                                stop=(kb == KBN - 1),
                            )
                    nc.vector.tensor_copy(out_sb[:, m, :], q_ps[:])

            wvt = load_wt(wv_d, nc.vector)
            for i in range(NPT):
                v_ps = pst.tile([P, 512], F32, tag="proj", bufs=2)
                for kb in range(KBN):
                    nc.tensor.matmul(
                        v_ps[:],
                        lhsT=yt[:, kb, i * P:(i + 1) * P],
                        rhs=wvt[:, kb, :],
                        start=(kb == 0),
                        stop=(kb == KBN - 1),
                    )
                # alternate engines so head 0's first e1t isn't queued
                # behind all eight V copies on ACT
                if i % 2 == 0:
                    nc.scalar.copy(
                        vv[:, i, :, 0:DK],
                        v_ps[:].rearrange("p (h d) -> p h d", h=FH),
                    )
                else:
                    nc.vector.tensor_copy(
                        vv[:, i, :, 0:DK],
                        v_ps[:].rearrange("p (h d) -> p h d", h=FH),
                    )

        # ---------------- attention: 3-stage pipeline over heads ----------
        e1p = ctx.enter_context(tc.tile_pool(name="e1p", bufs=12))
        e3p = ctx.enter_context(tc.tile_pool(name="e3p", bufs=18))
        e2p = ctx.enter_context(tc.tile_pool(name="e2p", bufs=18))
        r1p = ctx.enter_context(tc.tile_pool(name="r1p", bufs=2))
        r1f = ctx.enter_context(tc.tile_pool(name="r1f", bufs=2))
        smp = ctx.enter_context(tc.tile_pool(name="smp", bufs=24))
        stp = ctx.enter_context(tc.tile_pool(name="stp", bufs=2, space="PSUM"))
        z1p = ctx.enter_context(tc.tile_pool(name="z1p", bufs=1, space="PSUM"))
        opp = ctx.enter_context(tc.tile_pool(name="opp", bufs=2, space="PSUM"))

        e1_tiles = {}
        e3_tiles = {}
        e2_tiles = {}
        z1_tiles = {}
        r1f_tiles = {}
        o_ps_tiles = {}

        def st_e1(h, kb):
            """S^T block matmuls + e1t = exp(ST/8) (single ACT op / kb)."""
            hb, ho = h // 2, (h % 2) * DK
            st = stp.tile([P, N], F32, tag="st")
            for ch in range(2):
                nc.tensor.matmul(
                    st[:, ch * 512:(ch + 1) * 512],
                    lhsT=kt[ho:ho + DK, hb, kb * P:(kb + 1) * P],
                    rhs=qt[ho:ho + DK, hb, ch * 512:(ch + 1) * 512],
                    start=True,
                    stop=True,
                )
            e1 = e1p.tile([P, N], BF16, tag="e1")
            nc.scalar.activation(e1[:], st[:], EXP, scale=0.125)
            e1_tiles[h].append(e1)

        def z1_mms(h, kb):
            for ch, z1t in ((0, z1_tiles[h][0]), (1, z1_tiles[h][1])):
                nc.tensor.matmul(
                    z1t[0:1, :],
                    lhsT=ones_col[:, 0:1],
                    rhs=e1_tiles[h][kb][:, ch * 512:(ch + 1) * 512],
                    start=(kb == 0),
                    stop=(kb == KBN - 1),
                )

        def phase1_tail(h):
            """r1 = approx 1/Z1, broadcast across partitions (Pool, ~free)."""
            z1a, z1b = z1_tiles.pop(h)
            r1_sb = r1p.tile([1, N], BF16, tag="r1")
            _recip_fast(nc, r1_sb[0:1, 0:512], z1a[0:1, :])
            _recip_fast(nc, r1_sb[0:1, 512:N], z1b[0:1, :])
            r1full = r1f.tile([P, N], BF16, tag="r1f")
            nc.gpsimd.partition_broadcast(r1full[:], r1_sb[0:1, :])
            r1f_tiles[h] = r1full

        def dist_e3(h, kb):
            """dist^T = e1t * r1 (in place, bf16), e3t = exp(dist^T)."""
            e1 = e1_tiles[h][kb]
            eng = nc.gpsimd if kb in DIST_POOL_KBS else nc.vector
            eng.tensor_mul(e1[:], e1[:], r1f_tiles[h][:])
            e3 = e3p.tile([P, N], BF16, tag="e3")
            nc.scalar.activation(e3[:], e1[:], EXP)
            e3_tiles[h].append(e3)

        def e2_block(h):
            for kb in range(KBN):
                e2 = e2p.tile([P, N], BF16, tag="e2")
                if h == FH - 1:
                    # last head: ACT is idle during the drain, and keeping
                    # e2t off DVE unblocks the final O block sooner
                    nc.scalar.activation(e2[:], e1_tiles[h][kb][:], EXP,
                                         scale=-1.0)
                else:
                    _recip_fast(nc, e2[:], e3_tiles[h][kb][:])
                e2_tiles[h].append(e2)

        def o_block(h, qb):
            """Direct-form O matmuls, both branches in ONE psum bank:
            the first matmul's start=True zeroes the whole 2KB zero-region,
            everything after accumulates. e3 branch cols 0:65 -> att,
            e2 branch cols 65:130 -> catt."""
            o_ps = opp.tile([P, 2 * (DK + 1)], F32, tag="o")
            for kb in range(KBN):
                nc.tensor.matmul(
                    o_ps[:, 0:DK + 1],
                    lhsT=e3_tiles[h][kb][:, qb * P:(qb + 1) * P],
                    rhs=vv[:, kb, h, :],
                    start=(kb == 0),
                    stop=False,
                    skip_group_check=True,
                )
                nc.tensor.matmul(
                    o_ps[:, DK + 1:2 * (DK + 1)],
                    lhsT=e2_tiles[h][kb][:, qb * P:(qb + 1) * P],
                    rhs=vv[:, kb, h, :],
                    start=False,
                    stop=(kb == KBN - 1),
                    skip_group_check=True,
                )
            o_ps_tiles[(h, qb)] = o_ps

        def o_norm(h, qb):
            o_ps = o_ps_tiles.pop((h, qb))
            rr = smp.tile([P, 2], F32, tag="rr")
            nc.vector.reciprocal(rr[:, 0:1], o_ps[:, DK:DK + 1])
            nc.vector.reciprocal(rr[:, 1:2], o_ps[:, 2 * DK + 1:2 * DK + 2])
            cols = slice(h * DK, (h + 1) * DK)
            nc.vector.tensor_scalar_mul(
                att_sb[:, qb, cols], o_ps[:, 0:DK], rr[:, 0:1]
            )
            nc.vector.tensor_scalar_mul(
                catt_sb[:, qb, cols], o_ps[:, DK + 1:2 * DK + 1], rr[:, 1:2]
            )
            if h == FH - 1:
                nc.sync.dma_start(out=att_d[qb * P:(qb + 1) * P, :],
                                  in_=att_sb[:, qb, :])
                nc.sync.dma_start(out=catt_d[qb * P:(qb + 1) * P, :],
                                  in_=catt_sb[:, qb, :])

        for it in range(FH + 2):
            a, b, c = it - 2, it - 1, it
            if 0 <= b < FH:
                e3_tiles[b] = []
                e2_tiles[b] = []
            if c < FH:
                e1_tiles[c] = []
            for s in range(KBN):
                if c < FH:
                    st_e1(c, s)
                if s == 0 and 0 <= b < FH:
                    # deferred tail of head b's phase1: its last Z1 matmuls
                    # land after this iter's first ST block so the previous
                    # iteration's PE stream never stalls on the last e1t.
                    z1_mms(b, KBN - 1)
                    phase1_tail(b)
                if c < FH and s >= 1:
                    if s == 1:
                        z1_tiles[c] = (
                            z1p.tile([1, 512], F32, tag="z1a", name="z1a"),
                            z1p.tile([1, 512], F32, tag="z1b", name="z1b"),
                        )
                    z1_mms(c, s - 1)
                if 0 <= b < FH:
                    dist_e3(b, s)
                if a >= 0:
                    o_block(a, s)
                    if s >= 1:
                        o_norm(a, s - 1)
            if a >= 0:
                o_norm(a, KBN - 1)
                del e3_tiles[a], e2_tiles[a]
            if 0 <= b < FH:
                e2_block(b)
                del e1_tiles[b]

    nc.finalize()
    return nc


_NC_CACHE = {}


def _get_nc():
    if "nc" not in _NC_CACHE:
        _NC_CACHE["nc"] = build_nc()
    return _NC_CACHE["nc"]


def _make_in_maps(x, y, Wq, Wk, Wv):
    x = np.ascontiguousarray(np.asarray(x, dtype=np.float32))
    y = np.ascontiguousarray(np.asarray(y, dtype=np.float32))
    Wq = np.ascontiguousarray(np.asarray(Wq, dtype=np.float32))
    Wk = np.ascontiguousarray(np.asarray(Wk, dtype=np.float32))
    Wv = np.ascontiguousarray(np.asarray(Wv, dtype=np.float32))
    in_maps = []
    for c in range(8):
        b, h0 = c // 2, (c % 2) * 8
        rows = slice(h0 * DK, h0 * DK + NF)
        in_maps.append({
            "x": x[b],
            "y": y[b],
            "wq": np.ascontiguousarray(Wq[rows]),
            "wk": np.ascontiguousarray(Wk[rows]),
            "wv": np.ascontiguousarray(Wv[rows]),
        })
    return in_maps


def run_cores(x, y, Wq, Wk, Wv, trace=False, tmpdir=None):
    nc = _get_nc()
    res = run_bass_kernel_spmd(
        nc, _make_in_maps(x, y, Wq, Wk, Wv), core_ids=list(range(8)),
        trace=trace, tmpdir=tmpdir,
    )
    B = 4
    c_att = np.empty((B, N, 2 * NF), dtype=np.float32)
    att = np.empty((B, N, 2 * NF), dtype=np.float32)
    for c, r in enumerate(res.results):
        b, cols = c // 2, slice((c % 2) * NF, (c % 2) * NF + NF)
        c_att[b][:, cols] = r["catt"]
        att[b][:, cols] = r["att"]
    return (c_att, att), res


def kernel(x, y, Wq, Wk, Wv):
    out, _ = run_cores(x, y, Wq, Wk, Wv)
    return out

